# revision 20
# baseline (speedup 1.0000x reference)
"""GCN 2-layer kernel on 8 Trainium2 NeuronCores (Bass/Tile).

Sharding: core m owns dest rows [m*R, (m+1)*R). Edges partitioned by dest
row (core), then sub-chunk, then source chunk (8 GPSIMD core-groups per NC),
sorted by dest row. Per layer the SpMM is:
  - table (128, R) in SBUF: partition 16g+f = feature f of node chunk g
    (chunk g == core g's row shard, delivered by chunked partition-axis
    AllGather overlapped with the producer phase)
  - ap_gather (GPSIMD): per-group edge-ordered gather from table
  - GPSIMD elementwise multiply by edge values (bf16 stream)
  - DVE prefix scan (fp32 state) per partition
  - ap_gather #2: prefix values at per-row end positions
  - DVE shifted subtract -> per-(row,group) segment sums
  - TensorE one-hot SEL matmul: sum 8 group partials -> (feat, rows)
Layer 1 fuses the h @ W2 matmul per sub (h never hits DRAM); layer 2 fuses
the (7, rows) -> (rows, 7) transposes; log_softmax on row-major tiles.
x / W1 / W2 / edge values / h are bf16 (validated ~2e-3 rel err); the
layer-2 z path stays fp32.
"""

import sys

for p in ("/opt/trn_rl_repo",):
    if p not in sys.path:
        sys.path.insert(0, p)

import numpy as np
import ml_dtypes

import concourse.bass as bass
import concourse.mybir as mybir
import concourse.tile as tile
from concourse import bacc, library_config

F32 = mybir.dt.float32
BF16 = mybir.dt.bfloat16
I16 = mybir.dt.int16
BF16_NP = ml_dtypes.bfloat16


class Cfg:
    def __init__(self, N, E, IN, HID, OUT, SUBS, NW, NCH):
        self.N = N
        self.E = E
        self.IN = IN
        self.HID = HID
        self.OUT = OUT
        self.C = 8
        self.R = N // 8
        self.SUBS = SUBS
        self.SUBROWS = self.R // SUBS
        self.EXT = -(-(self.SUBROWS + 1) // 32) * 32
        self.NW = NW
        self.WCOL = self.R // NW
        self.NCH = NCH
        self.KC = -(-IN // 128)
        self.KLAST = IN - (self.KC - 1) * 128
        self.TCH = -(-self.SUBROWS // 128)      # transpose chunks per sub
        self.TPP = self.SUBROWS // self.TCH     # rows per transpose tile
        self.NT = SUBS * self.TCH               # total transpose tiles
        assert self.R % SUBS == 0 and self.R % NW == 0
        assert self.WCOL <= 512 and self.SUBROWS <= 512
        assert NW % NCH == 0 and SUBS % NCH == 0
        assert self.SUBROWS % self.TCH == 0 and self.TPP <= 128


FULL = Cfg(N=100_000, E=3_200_000, IN=1433, HID=16, OUT=7, SUBS=50, NW=25, NCH=5)


def prepare(x, adj_row, adj_col, adj_val, W1, b1, W2, b2, cfg):
    """Host preprocessing: build per-core input maps (pure numpy)."""
    N, E, R, C = cfg.N, cfg.E, cfg.R, cfg.C
    SUBS, SUBROWS, EXT = cfg.SUBS, cfg.SUBROWS, cfg.EXT
    KC, NW, WCOL = cfg.KC, cfg.NW, cfg.WCOL

    core = adj_row // R
    grp = adj_col // R
    sub = (adj_row - core * R) // SUBROWS
    order = np.lexsort((adj_row, grp, sub, core))
    r_s = adj_row[order]
    c_s = adj_col[order]
    v_s = adj_val[order]

    key = (core[order] * SUBS + sub[order]) * C + grp[order]
    ncell = C * SUBS * C
    starts = np.searchsorted(key, np.arange(ncell))
    ends = np.searchsorted(key, np.arange(ncell) + 1)
    cnt = (ends - starts).reshape(C, SUBS, C)

    # pad stream lengths to %32 so every packed idx-slice offset (L/16 cols
    # of int16) stays 4-byte aligned -- the ap_gather ucode requires it
    glk = cnt.max(axis=(0, 2)) + 1
    glk = (-(-glk // 32) * 32).astype(np.int64)
    glk = np.maximum(glk, 32)
    gtot = int(glk.sum())
    offs = np.concatenate([[0], np.cumsum(glk)])

    # per-sub packed idx stream widths (in 16-wrapped columns)
    wks = [int(glk[k]) // 16 + EXT // 16 for k in range(SUBS)]
    po = np.concatenate([[0], np.cumsum(wks)])
    ixw_tot = int(po[-1])

    # shared weights/constants
    xt = np.zeros((KC * 128, R * C), np.float32)
    xt[: cfg.IN] = x.T
    w1p = np.zeros((KC * 128, cfg.HID), np.float32)
    w1p[: cfg.IN] = W1
    w1s_h = np.ascontiguousarray(
        w1p.reshape(KC, 128, cfg.HID).transpose(1, 0, 2)
    ).astype(BF16_NP)
    sel1 = np.zeros((128, 16), np.float32)
    sel1[np.arange(128), np.arange(128) % 16] = 1.0
    sel2 = sel1.copy()
    for p_ in range(128):
        if p_ % 16 >= cfg.OUT:
            sel2[p_, p_ % 16] = 0.0
    id16 = np.eye(16, dtype=np.float32)

    in_maps = []
    for m in range(C):
        gidx = np.zeros((C, gtot), np.int16)
        valr = np.zeros((C, gtot), np.float32)
        eidx = np.zeros((C, SUBS * EXT), np.int16)
        for k in range(SUBS):
            o = offs[k]
            for g in range(C):
                ci = (m * SUBS + k) * C + g
                s, e = starts[ci], ends[ci]
                n = e - s
                gidx[g, o + 1 : o + 1 + n] = (c_s[s:e] - g * R).astype(np.int16)
                valr[g, o + 1 : o + 1 + n] = v_s[s:e]
                rows_rel = r_s[s:e] - (m * R + k * SUBROWS)
                pos = np.searchsorted(rows_rel, np.arange(SUBROWS), side="right")
                ex = np.zeros(EXT, np.int16)
                ex[1 : 1 + SUBROWS] = pos.astype(np.int16)
                ex[1 + SUBROWS :] = pos[-1] if SUBROWS else 0
                eidx[g, k * EXT : (k + 1) * EXT] = ex

        def wrap16(a):
            Cg, L = a.shape
            o_ = np.zeros((128, L // 16), a.dtype)
            for g in range(Cg):
                o_[16 * g : 16 * g + 16] = a[g].reshape(-1, 16).T
            return o_

        gidx_w = wrap16(gidx)
        eidx_w = wrap16(eidx)
        # pack per-sub: [gidx slice | eidx slice]
        ixp = np.zeros((128, ixw_tot), np.int16)
        for k in range(SUBS):
            L16 = int(glk[k]) // 16
            a = int(po[k])
            ixp[:, a : a + L16] = gidx_w[:, int(offs[k]) // 16 : int(offs[k + 1]) // 16]
            ixp[:, a + L16 : a + wks[k]] = eidx_w[
                :, k * EXT // 16 : (k + 1) * EXT // 16
            ]
        valrep = np.repeat(valr, 16, axis=0).astype(BF16_NP)

        xm = xt[:, m * R : (m + 1) * R]
        xh = np.ascontiguousarray(
            xm.reshape(KC, 128, NW, WCOL).transpose(2, 1, 0, 3)
        ).astype(BF16_NP)

        in_maps.append(
            dict(
                xh=xh,
                w1s=w1s_h,
                b1=np.ascontiguousarray(b1.reshape(cfg.HID, 1), dtype=np.float32),
                w2=np.ascontiguousarray(W2, dtype=np.float32).astype(BF16_NP),
                b2=np.ascontiguousarray(b2.reshape(cfg.OUT, 1), dtype=np.float32),
                ixp=ixp,
                vrp=valrep,
                sel1=sel1.astype(BF16_NP),
                sel2=sel2,
                id16=id16,
            )
        )
    return in_maps, glk, offs, po, wks


def build(cfg, glk, offs, po, wks, dbg=False):
    nc = bacc.Bacc("TRN2", target_bir_lowering=False, debug=False, num_devices=cfg.C)
    R, HID, OUT, SUBS, EXT = cfg.R, cfg.HID, cfg.OUT, cfg.SUBS, cfg.EXT
    NW, WCOL, NCH = cfg.NW, cfg.WCOL, cfg.NCH
    KC, KLAST = cfg.KC, cfg.KLAST
    SUBROWS, TCH, TPP, NT = cfg.SUBROWS, cfg.TCH, cfg.TPP, cfg.NT
    gtot = int(glk.sum())
    glkmax = int(max(glk))
    ixw_tot = int(po[-1])
    ixw_max = max(wks)
    WCH = NW // NCH            # windows per ag chunk
    SCH = SUBS // NCH          # subs per ag chunk
    CCOL = R // NCH            # columns per ag chunk

    xh = nc.dram_tensor("xh", [NW, 128, KC, WCOL], BF16, kind="ExternalInput").ap()
    w1s_d = nc.dram_tensor("w1s", [128, KC, HID], BF16, kind="ExternalInput").ap()
    b1 = nc.dram_tensor("b1", [HID, 1], F32, kind="ExternalInput").ap()
    w2_d = nc.dram_tensor("w2", [HID, OUT], BF16, kind="ExternalInput").ap()
    b2 = nc.dram_tensor("b2", [OUT, 1], F32, kind="ExternalInput").ap()
    ixp_d = nc.dram_tensor("ixp", [128, ixw_tot], I16, kind="ExternalInput").ap()
    vrp_d = nc.dram_tensor("vrp", [128, gtot], BF16, kind="ExternalInput").ap()
    sel1_d = nc.dram_tensor("sel1", [128, 16], BF16, kind="ExternalInput").ap()
    sel2_d = nc.dram_tensor("sel2", [128, 16], F32, kind="ExternalInput").ap()
    id16_d = nc.dram_tensor("id16", [16, 16], F32, kind="ExternalInput").ap()
    out = nc.dram_tensor("out", [R, OUT], F32, kind="ExternalOutput").ap()
    if dbg:
        dbg_tab = nc.dram_tensor("dbg_tab", [128, R], F32, kind="ExternalOutput").ap()
        dbg_tab2 = nc.dram_tensor("dbg_tab2", [128, R], F32, kind="ExternalOutput").ap()
        dbg_zr = nc.dram_tensor(
            "dbg_zr", [cfg.TPP, cfg.NT, OUT], F32, kind="ExternalOutput"
        ).ap()
        dbg_h = nc.dram_tensor("dbg_h", [HID, R], BF16, kind="ExternalOutput").ap()

    rg = [list(range(cfg.C))]

    with tile.TileContext(nc) as tc:
        with (
            tc.tile_pool(name="const", bufs=1) as cpool,
            tc.tile_pool(name="tab", bufs=1) as tabpool,
            tc.tile_pool(name="io", bufs=3) as iopool,
            tc.tile_pool(name="psA", bufs=2, space="PSUM") as papool,
            tc.tile_pool(name="psB", bufs=2, space="PSUM") as pbpool,
            tc.tile_pool(name="psZ", bufs=2, space="PSUM") as pzpool,
            tc.tile_pool(name="psT", bufs=2, space="PSUM") as ptpool,
            tc.tile_pool(name="dram", bufs=1, space="DRAM") as dpool,
        ):
            nc.gpsimd.load_library(library_config.ap_gather)

            # ---- consts
            w1s = cpool.tile([128, KC, HID], BF16)
            nc.sync.dma_start(w1s[:], w1s_d[:])
            b1s = cpool.tile([HID, 1], F32)
            nc.sync.dma_start(b1s[:], b1[:])
            w2s = cpool.tile([HID, OUT], BF16)
            nc.sync.dma_start(w2s[:], w2_d[:])
            b2s = cpool.tile([OUT, 1], F32)
            nc.sync.dma_start(b2s[:], b2[:])
            sel1s = cpool.tile([128, 16], BF16)
            nc.sync.dma_start(sel1s[:], sel1_d[:])
            sel2s = cpool.tile([128, 16], F32)
            nc.sync.dma_start(sel2s[:], sel2_d[:])
            id16s = cpool.tile([16, 16], F32)
            nc.sync.dma_start(id16s[:], id16_d[:])
            onesb = cpool.tile([128, glkmax], F32)
            nc.vector.memset(onesb[:], 1.0)

            table = tabpool.tile([128, R], F32)
            table2 = tabpool.tile([128, R], F32)
            zr = tabpool.tile([TPP, NT, OUT], F32)

            AGSPACE = "Local"
            ag1_in = [dpool.tile([HID, CCOL], F32, name=f"ag1i{c}") for c in range(NCH)]
            ag1_out = [
                dpool.tile([128, CCOL], F32, name=f"ag1o{c}", addr_space=AGSPACE)
                for c in range(NCH)
            ]
            ag2_in = [dpool.tile([16, CCOL], F32, name=f"ag2i{c}") for c in range(NCH)]
            ag2_out = [
                dpool.tile([128, CCOL], F32, name=f"ag2o{c}", addr_space=AGSPACE)
                for c in range(NCH)
            ]

            # ---- phase A: (X @ W1)^T windows + chunked AllGather -> table
            scA = nc.named_scope("denseA")
            scA.__enter__()
            with tc.tile_pool(name="xw", bufs=2) as xpool:
                z9 = xpool.tile([16 - OUT, CCOL], F32)
                nc.vector.memset(z9[:], 0.0)
                for c in range(NCH):
                    nc.scalar.dma_start(ag2_in[c][OUT:16, :], z9[:])
                for w in range(NW):
                    xw = xpool.tile([128, KC, WCOL], BF16, tag="xw")
                    nc.sync.dma_start(xw[:], xh[w])
                    pa = papool.tile([HID, WCOL], F32, tag="pa")
                    for k in range(KC):
                        kp = 128 if k < KC - 1 else KLAST
                        nc.tensor.matmul(
                            pa[:],
                            w1s[:kp, k, :],
                            xw[:kp, k, :],
                            start=(k == 0),
                            stop=(k == KC - 1),
                        )
                    st1 = iopool.tile([HID, WCOL], F32, tag="st1")
                    nc.scalar.copy(st1[:], pa[:])
                    c = w // WCH
                    nc.scalar.dma_start(
                        ag1_in[c][:, (w % WCH) * WCOL : (w % WCH + 1) * WCOL], st1[:]
                    )
                    if w % WCH == WCH - 1:
                        nc.gpsimd.collective_compute(
                            "AllGather",
                            mybir.AluOpType.bypass,
                            ins=[ag1_in[c].opt()],
                            outs=[ag1_out[c].opt()],
                            replica_groups=rg,
                        )
                        nc.sync.dma_start(
                            table[:, c * CCOL : (c + 1) * CCOL], ag1_out[c][:]
                        )
            scA.__exit__(None, None, None)

            # ---- spmm over subs
            def spmm(layer, tbl, scope):
                sc = nc.named_scope(scope)
                sc.__enter__()
                with tc.tile_pool(name=f"st{layer}", bufs=3) as spool:
                    for k in range(SUBS):
                        L = int(glk[k])
                        o = int(offs[k])
                        a = int(po[k])
                        wk = wks[k]
                        ix = spool.tile([128, ixw_max], I16, tag="ix")
                        nc.sync.dma_start(ix[:, :wk], ixp_d[:, a : a + wk])
                        vr = spool.tile([128, glkmax], BF16, tag="vr")
                        nc.scalar.dma_start(vr[:, :L], vrp_d[:, o : o + L])
                        gath = spool.tile([128, glkmax], F32, tag="gath")
                        nc.gpsimd.ap_gather(
                            gath[:, :L].rearrange("c (n d) -> c n d", d=1),
                            tbl[:].rearrange("c (n d) -> c n d", d=1),
                            ix[:, : L // 16],
                            channels=128,
                            num_elems=R,
                            d=1,
                            num_idxs=L,
                        )
                        nc.gpsimd.tensor_mul(gath[:, :L], gath[:, :L], vr[:, :L])
                        so = spool.tile([128, glkmax], F32, tag="so")
                        nc.vector.tensor_tensor_scan(
                            so[:, :L],
                            onesb[:, :L],
                            gath[:, :L],
                            0.0,
                            mybir.AluOpType.mult,
                            mybir.AluOpType.add,
                        )
                        rxt = spool.tile([128, EXT], F32, tag="rxt")
                        nc.gpsimd.ap_gather(
                            rxt[:].rearrange("c (n d) -> c n d", d=1),
                            so[:, :L].rearrange("c (n d) -> c n d", d=1),
                            ix[:, L // 16 : wk],
                            channels=128,
                            num_elems=L,
                            d=1,
                            num_idxs=EXT,
                        )
                        ddt = BF16 if layer == 1 else F32
                        dd = spool.tile([128, EXT], ddt, tag="dd")
                        nc.vector.tensor_sub(
                            dd[:, :SUBROWS], rxt[:, 1 : SUBROWS + 1], rxt[:, :SUBROWS]
                        )
                        pb = pbpool.tile([16, SUBROWS], F32, tag="pb")
                        sel = sel1s if layer == 1 else sel2s
                        nf = HID if layer == 1 else OUT
                        nc.tensor.matmul(
                            pb[:nf], sel[:, :nf], dd[:, :SUBROWS], start=True, stop=True
                        )
                        if layer == 1:
                            sto = iopool.tile([HID, SUBROWS], BF16, tag="sto")
                            nc.scalar.activation(
                                sto[:],
                                pb[:HID],
                                mybir.ActivationFunctionType.Relu,
                                bias=b1s[:],
                            )
                            pz = pzpool.tile([OUT, SUBROWS], F32, tag="pz")
                            nc.tensor.matmul(pz[:], w2s[:], sto[:], start=True, stop=True)
                            stz = iopool.tile([OUT, SUBROWS], F32, tag="stz")
                            nc.scalar.copy(stz[:], pz[:])
                            if dbg:
                                nc.sync.dma_start(
                                    dbg_h[:, k * SUBROWS : (k + 1) * SUBROWS], sto[:]
                                )
                            c = k // SCH
                            nc.scalar.dma_start(
                                ag2_in[c][
                                    :OUT, (k % SCH) * SUBROWS : (k % SCH + 1) * SUBROWS
                                ],
                                stz[:],
                            )
                            if k % SCH == SCH - 1:
                                scg = nc.named_scope("ag2")
                                scg.__enter__()
                                nc.gpsimd.collective_compute(
                                    "AllGather",
                                    mybir.AluOpType.bypass,
                                    ins=[ag2_in[c].opt()],
                                    outs=[ag2_out[c].opt()],
                                    replica_groups=rg,
                                )
                                nc.scalar.dma_start(
                                    table2[:, c * CCOL : (c + 1) * CCOL], ag2_out[c][:]
                                )
                                scg.__exit__(None, None, None)
                        else:
                            sto2 = iopool.tile([OUT, SUBROWS], F32, tag="sto2")
                            nc.scalar.activation(
                                sto2[:],
                                pb[:OUT],
                                mybir.ActivationFunctionType.Identity,
                                bias=b2s[:],
                            )
                            zrp = ptpool.tile([TPP, TCH, OUT], F32, tag="zrp")
                            for h in range(TCH):
                                nc.tensor.matmul(
                                    zrp[:, h, :],
                                    sto2[:, h * TPP : (h + 1) * TPP],
                                    id16s[:OUT, :OUT],
                                    is_transpose=True,
                                    start=True,
                                    stop=True,
                                )
                            nc.scalar.copy(zr[:, k * TCH : (k + 1) * TCH, :], zrp[:])
                sc.__exit__(None, None, None)

            spmm(1, table, "spmm1")
            spmm(2, table2, "spmm2")
            if dbg:
                nc.sync.dma_start(dbg_tab[:], table[:])
                nc.sync.dma_start(dbg_tab2[:], table2[:])
                nc.sync.dma_start(dbg_zr[:], zr[:])

            # ---- log_softmax on zr (TPP, NT, OUT); row = 125*t + p
            scS = nc.named_scope("softmax")
            scS.__enter__()
            mx = tabpool.tile([TPP, NT], F32)
            nc.vector.tensor_reduce(
                mx[:], zr[:], axis=mybir.AxisListType.X, op=mybir.AluOpType.max
            )
            zs = tabpool.tile([TPP, NT, OUT], F32)
            for j in range(OUT):
                nc.vector.tensor_sub(zs[:, :, j], zr[:, :, j], mx[:])
            ex = tabpool.tile([TPP, NT, OUT], F32)
            nc.scalar.activation(ex[:], zs[:], mybir.ActivationFunctionType.Exp)
            sm = tabpool.tile([TPP, NT], F32)
            nc.vector.tensor_reduce(
                sm[:], ex[:], axis=mybir.AxisListType.X, op=mybir.AluOpType.add
            )
            lg = tabpool.tile([TPP, NT], F32)
            nc.scalar.activation(lg[:], sm[:], mybir.ActivationFunctionType.Ln)
            for j in range(OUT):
                nc.vector.tensor_sub(zs[:, :, j], zs[:, :, j], lg[:])

            half = NT // 2
            nc.sync.dma_start(
                out[: half * TPP, :].rearrange("(t p) j -> p t j", p=TPP),
                zs[:, :half, :],
            )
            nc.scalar.dma_start(
                out[half * TPP :, :].rearrange("(t p) j -> p t j", p=TPP),
                zs[:, half:, :],
            )
            scS.__exit__(None, None, None)
    nc.compile()
    return nc


def kernel(x, adj_row, adj_col, adj_val, W1, b1, W2, b2):
    from concourse import bass_utils

    cfg = FULL
    in_maps, glk, offs, po, wks = prepare(
        np.asarray(x), np.asarray(adj_row), np.asarray(adj_col),
        np.asarray(adj_val), np.asarray(W1), np.asarray(b1),
        np.asarray(W2), np.asarray(b2), cfg,
    )
    nc = build(cfg, glk, offs, po, wks)
    res = bass_utils.run_bass_kernel_spmd(nc, in_maps, core_ids=list(range(cfg.C)))
    outs = [res.results[m]["out"] for m in range(cfg.C)]
    return np.concatenate(outs, axis=0)[: cfg.N]
